# revision 38
# baseline (speedup 1.0000x reference)
"""Trainium2 Bass kernel for nn_Attention_42348377538911.

3D attention: x [2, 128, 16, 16, 16] -> qkv 1x1x1 conv -> 4-head attention
over N=4096 positions (dim_head=32) -> out 1x1x1 conv.

Sharding: 8 cores = 2 batches x 4 heads (one (b, h) pair per core).
Each core computes its head's attention and a tensor-parallel partial of the
output projection; host sums the 4 partials per batch and adds b_out.

v3 layout:
  G-trick : host precomputes G = scale * w_q^T w_k (f64->f32).  On-chip
            u = G^T x (one 512-cycle matmul per i-tile), and
            simT chunk = x_chunk^T @ u_tile with full K=128 contraction --
            no separate q/k projections, no k evacuation, no tile_position
            bands.  x_sb doubles as the QK stationary.
  psum    : qk groups rotate through a 3-deep pool of [128, 1024] slots
            (2 banks each).  1024 is the widest exp op psum can support:
            two in-flight exp ops (one per engine) + the PE fill-ahead
            slot = 6 banks; wider ops serialize the pipeline.
  exp     : ACT (true exp -> bf16) vs DVE (i16 Schraudolph -> bf16 bits)
            chosen per-op by a greedy makespan balance over modeled costs.
  AV      : flipped orientation (stationary ex chunk, moving vt_aug 33-wide)
            with the softmax denominator via a ones column.
  v-proj  : bf16 (gpsimd converts x -> x8b; wvt bf16), 1 cycle/row.
  y-proj  : DVE 32x32 block-transpose + full-K matmuls against band-masked
            w_oT replicas; last tile's evac/DMA split across engines/queues.
  head    : x[0]/G race on separate HWDGE queues; 7 high-priority PE warmup
            matmuls keep the p-state ramping until real work lands.

PSUM: qk 6 banks + av 1 + y 1 = 8.
Error budget: Schraudolph bf16 on the DVE share of P (pre-softmax) + bf16
P/v/w_o; measured full-output rel err ~6.6e-3 vs the 2e-2 gate.
"""

import sys

import numpy as np

if "/opt/trn_rl_repo" not in sys.path:
    sys.path.insert(0, "/opt/trn_rl_repo")

HEADS = 4
DIM_HEAD = 32
B = 2
C = 128
N = 4096          # 16*16*16 spatial positions
NT = 512          # i-tile width
N_IT = N // NT    # 8 i-tiles
N_GRP = 16        # 2-chunk groups per i-tile
GW = 2            # chunks per group

# Schraudolph exp in bf16-bit domain: i16 = round(s * 128/ln2 + (127*128 - C))
SCH_A = 128.0 / float(np.log(2.0))
SCH_B = 127.0 * 128.0 - 5.0

# modeled per-op engine costs (ns) for the greedy exp balance
ACT_SINGLE = 1038.0
DVE_SINGLE = 1192.0

_cached = {}


def _build():
    import concourse.bacc as bacc
    import concourse.tile as tile
    import concourse.mybir as mybir
    from concourse.bass import ts

    f32 = mybir.dt.float32
    f32r = mybir.dt.float32r
    bf16 = mybir.dt.bfloat16
    i16 = mybir.dt.int16
    EXP = mybir.ActivationFunctionType.Exp
    COPY = mybir.ActivationFunctionType.Copy
    MULT = mybir.AluOpType.mult
    ADD = mybir.AluOpType.add
    DIV = mybir.AluOpType.divide

    nc = bacc.Bacc("TRN2", target_bir_lowering=False, debug=False, num_devices=8)
    x_d = nc.dram_tensor("x", [C, N], f32, kind="ExternalInput").ap()
    g_d = nc.dram_tensor("g_mat", [C, C], f32, kind="ExternalInput").ap()
    wvt_d = nc.dram_tensor("w_vT", [C, DIM_HEAD], bf16, kind="ExternalInput").ap()
    wort_d = nc.dram_tensor("w_oT_z", [C, 4 * C], bf16, kind="ExternalInput").ap()
    y_d = nc.dram_tensor("y", [C, N], f32, kind="ExternalOutput").ap()

    GCOLS = GW * NT  # columns per qk psum group

    with tile.TileContext(nc) as tc:
        with tc.tile_pool(name="sing", bufs=1) as sing:
            g_sb = sing.tile([C, C], f32r)
            wvt = sing.tile([C, DIM_HEAD], bf16)
            wort = sing.tile([C, 4, C], bf16)  # band-masked w_oT per pb
            x_sb = [sing.tile([C, NT], f32r, tag=f"x{cx}", name=f"x{cx}")
                    for cx in range(N_IT)]
            x8b = [sing.tile([C, NT], bf16, tag=f"x8b{cx}", name=f"x8b{cx}")
                   for cx in range(N_IT)]
            u_rt = [sing.tile([128, NT], f32r, tag=f"urt{it}", name=f"urt{it}")
                    for it in range(N_IT)]
            vt_aug = sing.tile([128, 32, 33], bf16)   # [j-in-chunk, chunk, d+1]
            scr = sing.tile([1, 64], f32)

            # x[0] and G race on separate HWDGE queues (SP / ACT); remaining
            # x chunks spread over the gpsimd SWDGE + both HWDGE queues.
            nc.sync.dma_start(x_sb[0], x_d[:, ts(0, NT)].bitcast(f32r))
            nc.scalar.dma_start(g_sb, g_d.bitcast(f32r))
            dma_engs = [nc.gpsimd, nc.sync, nc.gpsimd, nc.scalar]
            for cx in range(1, N_IT):
                dma_engs[cx % 4].dma_start(
                    x_sb[cx], x_d[:, ts(cx, NT)].bitcast(f32r))
            nc.sync.dma_start(wvt, wvt_d)
            nc.sync.dma_start(
                wort[:], wort_d.rearrange("p (b c) -> p b c", c=C))
            # PE warmup fodder with no DMA dependency (ramps the p-state and
            # keeps PE continuously busy until x[0]/G land)
            wsrc = sing.tile([128, 512], f32r)
            with tc.high_priority():
                nc.gpsimd.memset(wsrc[:].bitcast(f32), 0.0)
            # warm the ACT exp table while P0 runs
            nc.gpsimd.memset(scr, 0.0)
            nc.scalar.activation(scr, scr, EXP)
            nc.gpsimd.memset(vt_aug[:], 1.0)

            # ---------------- P1: attention ----------------
            with tc.tile_pool(name="exp", bufs=32) as ex_pool, \
                 tc.tile_pool(name="nrm", bufs=3) as nrm, \
                 tc.tile_pool(name="ysb", bufs=2) as ysb, \
                 tc.tile_pool(name="qkp", bufs=3, space="PSUM") as qkp, \
                 tc.tile_pool(name="avp", bufs=1, space="PSUM") as avp, \
                 tc.tile_pool(name="yps", bufs=1, space="PSUM") as yps:

                ex_tiles = [None] * N_IT
                avnT_tiles = [None] * N_IT

                # arrival-aware exp engine balance: exp(g) becomes available
                # when PE finishes QK(g) (~PROD ns apart in steady state);
                # assign to the engine with the earliest completion.
                PROD = 590.0
                bal = {"act": 8 * 612.0, "dve": 0.0}
                gcount = [0]

                def emit_qk_group(step, gk, cur):
                    jc0 = GW * gk
                    qk_ps = qkp.tile([128, GCOLS], f32, tag="qk")
                    ex_t = ex_pool.tile([128, GCOLS], bf16, tag="ex")
                    for r in range(GW):
                        jc = jc0 + r
                        nc.tensor.matmul(
                            qk_ps[:, ts(r, NT)],
                            lhsT=x_sb[jc // 4][:, ts(jc % 4, 128)],
                            rhs=u_rt[step],
                            start=True, stop=True)
                    gid = gcount[0]
                    gcount[0] += 1
                    # strict alternation: PE produces just-in-time (3 psum
                    # slots = no backlog), so consecutive same-engine groups
                    # starve the other engine for a full op.  Two mid-run
                    # flips to ACT (where DVE has accumulated slot backlog)
                    # equalize the 1038 vs 1192 ns per-op costs.
                    use_act = gid % 2 == 0 or gid in (57, 97)
                    if use_act:
                        nc.scalar.activation(ex_t, qk_ps, EXP)
                    else:
                        nc.vector.tensor_scalar(
                            ex_t[:, 0:GCOLS].bitcast(i16), qk_ps,
                            SCH_A, SCH_B, MULT, ADD)
                    cur.append(ex_t)

                # ---- fused P0 + QK/exp of i-tile 0 ----
                # u projections borrow the (idle until later) av and y banks
                # for psum; each u_rt[0]-ready pair unlocks i-tile 0 QK.
                warm = qkp.tile([128, GCOLS], f32, tag="qk")
                with tc.high_priority():
                    for _ in range(7):
                        nc.tensor.matmul(warm[:, 0:NT], lhsT=wsrc[:, 0:C],
                                         rhs=wsrc, start=True, stop=True)
                cur0 = []
                for it in range(N_IT):
                    pool = avp if it % 2 == 0 else yps
                    pu = pool.tile([128, NT], f32,
                                   tag="avy" if it % 2 == 0 else "yy")
                    nc.tensor.matmul(pu, lhsT=g_sb, rhs=x_sb[it],
                                     start=True, stop=True)
                    nc.scalar.activation(u_rt[it], pu, COPY)
                    nc.gpsimd.tensor_copy(x8b[it], x_sb[it].bitcast(f32))
                    emit_qk_group(0, 2 * it, cur0)
                    emit_qk_group(0, 2 * it + 1, cur0)
                ex_tiles[0] = cur0

                # ---- vT projection (borrows the av bank, pre-AV(0)) ----
                for half in range(2):
                    ps2 = avp.tile([128, NT], f32, tag="avy")
                    for jj in range(16):
                        jc = half * 16 + jj
                        nc.tensor.matmul(
                            ps2[:, ts(jj, 32)],
                            lhsT=x8b[jc // 4][:, ts(jc % 4, 128)],
                            rhs=wvt,
                            start=True, stop=True)
                    nc.scalar.activation(
                        vt_aug[:, half * 16:(half + 1) * 16, 0:DIM_HEAD],
                        ps2[:].rearrange("p (c d) -> p c d", d=DIM_HEAD),
                        COPY)

                for step in range(1, N_IT + 2):
                    do_qk = step < N_IT
                    do_av = 1 <= step <= N_IT
                    do_y = step >= 2
                    cur = [] if do_qk else None
                    if do_av:
                        av_ps = avp.tile([128, 4 * 33], f32, tag="avy")
                        prev_ex = ex_tiles[step - 1]
                    if do_y:
                        avnT_y = avnT_tiles[step - 2]
                        y_ps = yps.tile([128, NT], f32, tag="yy")

                    # ---- y evac + DMA for i-tile step-2 ----
                    # psum cols are (pb, fb, il); y_sb wants i = (fb, pb, il)
                    def emit_y_evac(it, y_ps):
                        y_sb = ysb.tile([128, NT], f32, tag="ysb")
                        yp_v = y_ps[:].rearrange(
                            "p (pb fb il) -> p pb fb il", pb=4, il=32)
                        if it == N_IT - 1:
                            # tail: split evac across ACT+DVE and DMA across
                            # two HWDGE queues so gen/transfer/sem overlap
                            nc.scalar.activation(
                                y_sb[:, 0:NT // 2].rearrange(
                                    "p (fb pb il) -> p pb fb il",
                                    pb=4, il=32),
                                yp_v[:, :, 0:2, :], COPY)
                            nc.vector.tensor_copy(
                                y_sb[:, NT // 2:NT].rearrange(
                                    "p (fb pb il) -> p pb fb il",
                                    pb=4, il=32),
                                yp_v[:, :, 2:4, :])
                            nc.sync.dma_start(
                                y_d[:, it * NT:it * NT + NT // 2],
                                y_sb[:, 0:NT // 2])
                            nc.scalar.dma_start(
                                y_d[:, it * NT + NT // 2:(it + 1) * NT],
                                y_sb[:, NT // 2:NT])
                        else:
                            nc.scalar.activation(
                                y_sb[:].rearrange(
                                    "p (fb pb il) -> p pb fb il",
                                    pb=4, il=32),
                                yp_v, COPY)
                            bal["act"] += 612.0
                            eng = nc.sync if it % 2 == 0 else nc.gpsimd
                            eng.dma_start(y_d[:, ts(it, NT)], y_sb)

                    # In drain steps (no QK) the y(t-2) matmuls + evac go
                    # FIRST so the y evac + DMA chain overlaps the AV tail
                    # instead of serializing after it.
                    if do_y and not do_qk:
                        for pb in range(4):
                            nc.tensor.matmul(
                                y_ps[:, ts(pb, 128)],
                                lhsT=wort[:, pb, :],
                                rhs=avnT_y,
                                start=True, stop=True)
                        emit_y_evac(step - 2, y_ps)

                    # Interleave per group: QK(t) + exp, AV(t-1) chunk-major
                    # accumulate, and one y(t-2) matmul -- keeps the in-order
                    # PE queue fed while exp drains the qk psum slots.
                    for gk in range(N_GRP + 1):
                        if do_qk and gk < N_GRP:
                            emit_qk_group(step, gk, cur)
                        gv = gk - 1  # av/y lag one group so QK leads
                        if gv < 0:
                            continue
                        if do_av:
                            # one av region (i-block) per 4 groups; each
                            # region's 32-matmul accumulation chain stays
                            # contiguous (the PE honours only one OPEN
                            # chain; self-contained QK/y matmuls between
                            # are fine)
                            ib = gv // 4
                            for m in range(8):
                                jc = (gv % 4) * 8 + m
                                ex_p = prev_ex[jc // 2]
                                r = jc % 2
                                nc.tensor.matmul(
                                    av_ps[:, 33 * ib:33 * ib + 33],
                                    lhsT=ex_p[:, r * NT + ib * 128:
                                              r * NT + (ib + 1) * 128],
                                    rhs=vt_aug[:, jc, :],
                                    start=(jc == 0), stop=(jc == 31))
                        if do_y and do_qk and gv % 4 == 3:
                            # band-masked stationary (zeros off-band) makes
                            # this a full-K matmul; one 128-col matmul per
                            # band pb covers all four i-blocks at once.
                            # psum column order is (pb, fb, il); the evac AP
                            # permutes back to i-order (fb, pb, il) for free.
                            pb = gv // 4
                            nc.tensor.matmul(
                                y_ps[:, ts(pb, 128)],
                                lhsT=wort[:, pb, :],
                                rhs=avnT_y,
                                start=True, stop=True)
                    if do_qk:
                        ex_tiles[step] = cur

                    if do_y and do_qk:
                        emit_y_evac(step - 2, y_ps)

                    # ---- normalize + transpose for i-tile step-1 ----
                    if do_av:
                        it = step - 1
                        ex_tiles[it] = None
                        av_v = av_ps[:].rearrange("p (b c) -> p b c", c=33)
                        rc = nrm.tile([128, 4], f32, tag="rc")
                        nc.vector.reciprocal(rc, av_v[:, :, 32])
                        avn = nrm.tile([128, 128], bf16, tag="avn")
                        nc.vector.tensor_tensor(
                            avn[:].rearrange("p (b c) -> p b c", c=DIM_HEAD),
                            av_v[:, :, 0:DIM_HEAD],
                            rc[:].unsqueeze(2).broadcast_to((128, 4, DIM_HEAD)),
                            MULT)
                        avnT = nrm.tile([128, 128], bf16, tag="avnT")
                        nc.vector.transpose(avnT, avn)
                        avnT_tiles[it] = avnT
                        bal["dve"] += 580.0

    nc.compile()
    return nc


def _get_nc():
    if "nc" not in _cached:
        _cached["nc"] = _build()
    return _cached["nc"]


def _make_in_maps(x, w_qkv, w_out):
    import ml_dtypes

    scale = DIM_HEAD ** -0.5
    in_maps = []
    for core in range(8):
        b, h = core // HEADS, core % HEADS
        w_q = w_qkv[h * DIM_HEAD:(h + 1) * DIM_HEAD, :].astype(np.float64)
        w_k = w_qkv[128 + h * DIM_HEAD:128 + (h + 1) * DIM_HEAD, :].astype(
            np.float64)
        w_v = w_qkv[256 + h * DIM_HEAD:256 + (h + 1) * DIM_HEAD, :]
        w_oT = w_out[:, h * DIM_HEAD:(h + 1) * DIM_HEAD].T  # [d, c]
        g_mat = (scale * (w_q.T @ w_k)).astype(np.float32)
        # band-masked replicas: band pb of slot pb holds w_oT, rest zero
        wortz = np.zeros((C, 4, C), np.float32)
        for pb in range(4):
            wortz[32 * pb:32 * pb + 32, pb, :] = w_oT
        in_maps.append({
            "x": np.ascontiguousarray(x[b].reshape(C, N)),
            "g_mat": np.ascontiguousarray(g_mat),
            "w_vT": np.ascontiguousarray(w_v.T.astype(ml_dtypes.bfloat16)),
            "w_oT_z": np.ascontiguousarray(
                wortz.reshape(C, 4 * C).astype(ml_dtypes.bfloat16)),
        })
    return in_maps


def _gather(results, b_out):
    y = np.zeros((B, C, N), dtype=np.float32)
    for core in range(8):
        y[core // HEADS] += results[core]["y"]
    y += b_out.astype(np.float32)[None, :, None]
    return y.reshape(B, C, 16, 16, 16)


def run(x, w_qkv, w_out, b_out, trace=False):
    from concourse.bass_utils import run_bass_kernel_spmd
    nc = _get_nc()
    in_maps = _make_in_maps(np.asarray(x), np.asarray(w_qkv), np.asarray(w_out))
    res = run_bass_kernel_spmd(nc, in_maps, core_ids=list(range(8)),
                               trace=trace)
    return _gather(res.results, np.asarray(b_out)), res


def kernel(x, w_qkv, w_out, b_out):
    y, _ = run(x, w_qkv, w_out, b_out)
    return y


# revision 39
# speedup vs baseline: 1.0043x; 1.0043x over previous
"""Trainium2 Bass kernel for nn_Attention_42348377538911.

3D attention: x [2, 128, 16, 16, 16] -> qkv 1x1x1 conv -> 4-head attention
over N=4096 positions (dim_head=32) -> out 1x1x1 conv.

Sharding: 8 cores = 2 batches x 4 heads (one (b, h) pair per core).
Each core computes its head's attention and a tensor-parallel partial of the
output projection; host sums the 4 partials per batch and adds b_out.

v3 layout:
  G-trick : host precomputes G = scale * w_q^T w_k (f64->f32).  On-chip
            u = G^T x (one 512-cycle matmul per i-tile), and
            simT chunk = x_chunk^T @ u_tile with full K=128 contraction --
            no separate q/k projections, no k evacuation, no tile_position
            bands.  x_sb doubles as the QK stationary.
  psum    : qk groups rotate through a 3-deep pool of [128, 1024] slots
            (2 banks each).  1024 is the widest exp op psum can support:
            two in-flight exp ops (one per engine) + the PE fill-ahead
            slot = 6 banks; wider ops serialize the pipeline.
  exp     : ACT (true exp -> bf16) vs DVE (i16 Schraudolph -> bf16 bits)
            chosen per-op by a greedy makespan balance over modeled costs.
  AV      : flipped orientation (stationary ex chunk, moving vt_aug 33-wide)
            with the softmax denominator via a ones column.
  v-proj  : bf16 (gpsimd converts x -> x8b; wvt bf16), 1 cycle/row.
  y-proj  : DVE 32x32 block-transpose + full-K matmuls against band-masked
            w_oT replicas; last tile's evac/DMA split across engines/queues.
  head    : x[0]/G race on separate HWDGE queues; 7 high-priority PE warmup
            matmuls keep the p-state ramping until real work lands.

PSUM: qk 6 banks + av 1 + y 1 = 8.
Error budget: Schraudolph bf16 on the DVE share of P (pre-softmax) + bf16
P/v/w_o; measured full-output rel err ~6.6e-3 vs the 2e-2 gate.
"""

import sys

import numpy as np

if "/opt/trn_rl_repo" not in sys.path:
    sys.path.insert(0, "/opt/trn_rl_repo")

HEADS = 4
DIM_HEAD = 32
B = 2
C = 128
N = 4096          # 16*16*16 spatial positions
NT = 512          # i-tile width
N_IT = N // NT    # 8 i-tiles
N_GRP = 16        # 2-chunk groups per i-tile
GW = 2            # chunks per group

# Schraudolph exp in bf16-bit domain: i16 = round(s * 128/ln2 + (127*128 - C))
SCH_A = 128.0 / float(np.log(2.0))
SCH_B = 127.0 * 128.0 - 5.0

# modeled per-op engine costs (ns) for the greedy exp balance
ACT_SINGLE = 1038.0
DVE_SINGLE = 1192.0

_cached = {}


def _build():
    import concourse.bacc as bacc
    import concourse.tile as tile
    import concourse.mybir as mybir
    from concourse.bass import ts

    f32 = mybir.dt.float32
    f32r = mybir.dt.float32r
    bf16 = mybir.dt.bfloat16
    i16 = mybir.dt.int16
    EXP = mybir.ActivationFunctionType.Exp
    COPY = mybir.ActivationFunctionType.Copy
    MULT = mybir.AluOpType.mult
    ADD = mybir.AluOpType.add
    DIV = mybir.AluOpType.divide

    nc = bacc.Bacc("TRN2", target_bir_lowering=False, debug=False, num_devices=8)
    x_d = nc.dram_tensor("x", [C, N], f32, kind="ExternalInput").ap()
    g_d = nc.dram_tensor("g_mat", [C, C], f32, kind="ExternalInput").ap()
    wvt_d = nc.dram_tensor("w_vT", [C, DIM_HEAD], bf16, kind="ExternalInput").ap()
    wort_d = nc.dram_tensor("w_oT_z", [C, 4 * C], bf16, kind="ExternalInput").ap()
    y_d = nc.dram_tensor("y", [C, N], f32, kind="ExternalOutput").ap()

    GCOLS = GW * NT  # columns per qk psum group

    with tile.TileContext(nc) as tc:
        with tc.tile_pool(name="sing", bufs=1) as sing:
            g_sb = sing.tile([C, C], f32r)
            wvt = sing.tile([C, DIM_HEAD], bf16)
            wort = sing.tile([C, 4, C], bf16)  # band-masked w_oT per pb
            x_sb = [sing.tile([C, NT], f32r, tag=f"x{cx}", name=f"x{cx}")
                    for cx in range(N_IT)]
            x8b = [sing.tile([C, NT], bf16, tag=f"x8b{cx}", name=f"x8b{cx}")
                   for cx in range(N_IT)]
            u_rt = [sing.tile([128, NT], f32r, tag=f"urt{it}", name=f"urt{it}")
                    for it in range(N_IT)]
            vt_aug = sing.tile([128, 32, 33], bf16)   # [j-in-chunk, chunk, d+1]
            scr = sing.tile([1, 64], f32)

            # x[0] and G race on separate HWDGE queues (SP / ACT); remaining
            # x chunks spread over the gpsimd SWDGE + both HWDGE queues.
            nc.sync.dma_start(x_sb[0], x_d[:, ts(0, NT)].bitcast(f32r))
            nc.scalar.dma_start(g_sb, g_d.bitcast(f32r))
            dma_engs = [nc.gpsimd, nc.sync, nc.gpsimd, nc.scalar]
            for cx in range(1, N_IT):
                dma_engs[cx % 4].dma_start(
                    x_sb[cx], x_d[:, ts(cx, NT)].bitcast(f32r))
            nc.sync.dma_start(wvt, wvt_d)
            nc.sync.dma_start(
                wort[:], wort_d.rearrange("p (b c) -> p b c", c=C))
            # PE warmup fodder with no DMA dependency (ramps the p-state and
            # keeps PE continuously busy until x[0]/G land)
            wsrc = sing.tile([128, 512], f32r)
            with tc.high_priority():
                nc.gpsimd.memset(wsrc[:].bitcast(f32), 0.0)
            # warm the ACT exp table while P0 runs
            nc.gpsimd.memset(scr, 0.0)
            nc.scalar.activation(scr, scr, EXP)
            nc.gpsimd.memset(vt_aug[:], 1.0)

            # ---------------- P1: attention ----------------
            with tc.tile_pool(name="exp", bufs=32) as ex_pool, \
                 tc.tile_pool(name="nrm", bufs=3) as nrm, \
                 tc.tile_pool(name="ysb", bufs=2) as ysb, \
                 tc.tile_pool(name="qkp", bufs=3, space="PSUM") as qkp, \
                 tc.tile_pool(name="avp", bufs=1, space="PSUM") as avp, \
                 tc.tile_pool(name="yps", bufs=1, space="PSUM") as yps:

                ex_tiles = [None] * N_IT
                avnT_tiles = [None] * N_IT

                # arrival-aware exp engine balance: exp(g) becomes available
                # when PE finishes QK(g) (~PROD ns apart in steady state);
                # assign to the engine with the earliest completion.
                PROD = 590.0
                bal = {"act": 8 * 612.0, "dve": 0.0}
                gcount = [0]

                def emit_qk_group(step, gk, cur):
                    jc0 = GW * gk
                    qk_ps = qkp.tile([128, GCOLS], f32, tag="qk")
                    ex_t = ex_pool.tile([128, GCOLS], bf16, tag="ex")
                    for r in range(GW):
                        jc = jc0 + r
                        nc.tensor.matmul(
                            qk_ps[:, ts(r, NT)],
                            lhsT=x_sb[jc // 4][:, ts(jc % 4, 128)],
                            rhs=u_rt[step],
                            start=True, stop=True)
                    gid = gcount[0]
                    gcount[0] += 1
                    # strict alternation: PE produces just-in-time (3 psum
                    # slots = no backlog), so consecutive same-engine groups
                    # starve the other engine for a full op.  Two mid-run
                    # flips to ACT (where DVE has accumulated slot backlog)
                    # equalize the 1038 vs 1192 ns per-op costs.
                    use_act = gid % 2 == 0 or gid in (57, 97)
                    if use_act:
                        nc.scalar.activation(ex_t, qk_ps, EXP)
                    else:
                        nc.vector.tensor_scalar(
                            ex_t[:, 0:GCOLS].bitcast(i16), qk_ps,
                            SCH_A, SCH_B, MULT, ADD)
                    cur.append(ex_t)

                # ---- fused P0 + QK/exp of i-tile 0 ----
                # u projections borrow the (idle until later) av and y banks
                # for psum; each u_rt[0]-ready pair unlocks i-tile 0 QK.
                warm = qkp.tile([128, GCOLS], f32, tag="qk")
                with tc.high_priority():
                    for _ in range(7):
                        nc.tensor.matmul(warm[:, 0:NT], lhsT=wsrc[:, 0:C],
                                         rhs=wsrc, start=True, stop=True)
                cur0 = []
                for it in range(N_IT):
                    pool = avp if it % 2 == 0 else yps
                    pu = pool.tile([128, NT], f32,
                                   tag="avy" if it % 2 == 0 else "yy")
                    nc.tensor.matmul(pu, lhsT=g_sb, rhs=x_sb[it],
                                     start=True, stop=True)
                    nc.scalar.activation(u_rt[it], pu, COPY)
                    nc.gpsimd.tensor_copy(x8b[it], x_sb[it].bitcast(f32))
                    emit_qk_group(0, 2 * it, cur0)
                    emit_qk_group(0, 2 * it + 1, cur0)
                ex_tiles[0] = cur0

                # ---- vT projection (borrows the av bank, pre-AV(0)) ----
                for half in range(2):
                    ps2 = avp.tile([128, NT], f32, tag="avy")
                    for jj in range(16):
                        jc = half * 16 + jj
                        nc.tensor.matmul(
                            ps2[:, ts(jj, 32)],
                            lhsT=x8b[jc // 4][:, ts(jc % 4, 128)],
                            rhs=wvt,
                            start=True, stop=True)
                    nc.vector.tensor_copy(
                        vt_aug[:, half * 16:(half + 1) * 16, 0:DIM_HEAD],
                        ps2[:].rearrange("p (c d) -> p c d", d=DIM_HEAD))

                for step in range(1, N_IT + 2):
                    do_qk = step < N_IT
                    do_av = 1 <= step <= N_IT
                    do_y = step >= 2
                    cur = [] if do_qk else None
                    if do_av:
                        av_ps = avp.tile([128, 4 * 33], f32, tag="avy")
                        prev_ex = ex_tiles[step - 1]
                    if do_y:
                        avnT_y = avnT_tiles[step - 2]
                        y_ps = yps.tile([128, NT], f32, tag="yy")

                    # ---- y evac + DMA for i-tile step-2 ----
                    # psum cols are (pb, fb, il); y_sb wants i = (fb, pb, il)
                    def emit_y_evac(it, y_ps):
                        y_sb = ysb.tile([128, NT], f32, tag="ysb")
                        yp_v = y_ps[:].rearrange(
                            "p (pb fb il) -> p pb fb il", pb=4, il=32)
                        if it == N_IT - 1:
                            # tail: split evac across ACT+DVE and DMA across
                            # two HWDGE queues so gen/transfer/sem overlap
                            nc.scalar.activation(
                                y_sb[:, 0:NT // 2].rearrange(
                                    "p (fb pb il) -> p pb fb il",
                                    pb=4, il=32),
                                yp_v[:, :, 0:2, :], COPY)
                            nc.vector.tensor_copy(
                                y_sb[:, NT // 2:NT].rearrange(
                                    "p (fb pb il) -> p pb fb il",
                                    pb=4, il=32),
                                yp_v[:, :, 2:4, :])
                            nc.sync.dma_start(
                                y_d[:, it * NT:it * NT + NT // 2],
                                y_sb[:, 0:NT // 2])
                            nc.scalar.dma_start(
                                y_d[:, it * NT + NT // 2:(it + 1) * NT],
                                y_sb[:, NT // 2:NT])
                        else:
                            nc.scalar.activation(
                                y_sb[:].rearrange(
                                    "p (fb pb il) -> p pb fb il",
                                    pb=4, il=32),
                                yp_v, COPY)
                            bal["act"] += 612.0
                            eng = nc.sync if it % 2 == 0 else nc.gpsimd
                            eng.dma_start(y_d[:, ts(it, NT)], y_sb)

                    # In drain steps (no QK) the y(t-2) matmuls + evac go
                    # FIRST so the y evac + DMA chain overlaps the AV tail
                    # instead of serializing after it.
                    if do_y and not do_qk:
                        for pb in range(4):
                            nc.tensor.matmul(
                                y_ps[:, ts(pb, 128)],
                                lhsT=wort[:, pb, :],
                                rhs=avnT_y,
                                start=True, stop=True)
                        emit_y_evac(step - 2, y_ps)

                    # Interleave per group: QK(t) + exp, AV(t-1) chunk-major
                    # accumulate, and one y(t-2) matmul -- keeps the in-order
                    # PE queue fed while exp drains the qk psum slots.
                    for gk in range(N_GRP + 1):
                        if do_qk and gk < N_GRP:
                            emit_qk_group(step, gk, cur)
                        gv = gk - 1  # av/y lag one group so QK leads
                        if gv < 0:
                            continue
                        if do_av:
                            # one av region (i-block) per 4 groups; each
                            # region's 32-matmul accumulation chain stays
                            # contiguous (the PE honours only one OPEN
                            # chain; self-contained QK/y matmuls between
                            # are fine)
                            ib = gv // 4
                            for m in range(8):
                                jc = (gv % 4) * 8 + m
                                ex_p = prev_ex[jc // 2]
                                r = jc % 2
                                nc.tensor.matmul(
                                    av_ps[:, 33 * ib:33 * ib + 33],
                                    lhsT=ex_p[:, r * NT + ib * 128:
                                              r * NT + (ib + 1) * 128],
                                    rhs=vt_aug[:, jc, :],
                                    start=(jc == 0), stop=(jc == 31))
                        if do_y and do_qk and gv % 4 == 3:
                            # band-masked stationary (zeros off-band) makes
                            # this a full-K matmul; one 128-col matmul per
                            # band pb covers all four i-blocks at once.
                            # psum column order is (pb, fb, il); the evac AP
                            # permutes back to i-order (fb, pb, il) for free.
                            pb = gv // 4
                            nc.tensor.matmul(
                                y_ps[:, ts(pb, 128)],
                                lhsT=wort[:, pb, :],
                                rhs=avnT_y,
                                start=True, stop=True)
                    if do_qk:
                        ex_tiles[step] = cur

                    if do_y and do_qk:
                        emit_y_evac(step - 2, y_ps)

                    # ---- normalize + transpose for i-tile step-1 ----
                    if do_av:
                        it = step - 1
                        ex_tiles[it] = None
                        av_v = av_ps[:].rearrange("p (b c) -> p b c", c=33)
                        rc = nrm.tile([128, 4], f32, tag="rc")
                        nc.vector.reciprocal(rc, av_v[:, :, 32])
                        avn = nrm.tile([128, 128], bf16, tag="avn")
                        nc.vector.tensor_tensor(
                            avn[:].rearrange("p (b c) -> p b c", c=DIM_HEAD),
                            av_v[:, :, 0:DIM_HEAD],
                            rc[:].unsqueeze(2).broadcast_to((128, 4, DIM_HEAD)),
                            MULT)
                        avnT = nrm.tile([128, 128], bf16, tag="avnT")
                        nc.vector.transpose(avnT, avn)
                        avnT_tiles[it] = avnT
                        bal["dve"] += 580.0

    nc.compile()
    return nc


def _get_nc():
    if "nc" not in _cached:
        _cached["nc"] = _build()
    return _cached["nc"]


def _make_in_maps(x, w_qkv, w_out):
    import ml_dtypes

    scale = DIM_HEAD ** -0.5
    in_maps = []
    for core in range(8):
        b, h = core // HEADS, core % HEADS
        w_q = w_qkv[h * DIM_HEAD:(h + 1) * DIM_HEAD, :].astype(np.float64)
        w_k = w_qkv[128 + h * DIM_HEAD:128 + (h + 1) * DIM_HEAD, :].astype(
            np.float64)
        w_v = w_qkv[256 + h * DIM_HEAD:256 + (h + 1) * DIM_HEAD, :]
        w_oT = w_out[:, h * DIM_HEAD:(h + 1) * DIM_HEAD].T  # [d, c]
        g_mat = (scale * (w_q.T @ w_k)).astype(np.float32)
        # band-masked replicas: band pb of slot pb holds w_oT, rest zero
        wortz = np.zeros((C, 4, C), np.float32)
        for pb in range(4):
            wortz[32 * pb:32 * pb + 32, pb, :] = w_oT
        in_maps.append({
            "x": np.ascontiguousarray(x[b].reshape(C, N)),
            "g_mat": np.ascontiguousarray(g_mat),
            "w_vT": np.ascontiguousarray(w_v.T.astype(ml_dtypes.bfloat16)),
            "w_oT_z": np.ascontiguousarray(
                wortz.reshape(C, 4 * C).astype(ml_dtypes.bfloat16)),
        })
    return in_maps


def _gather(results, b_out):
    y = np.zeros((B, C, N), dtype=np.float32)
    for core in range(8):
        y[core // HEADS] += results[core]["y"]
    y += b_out.astype(np.float32)[None, :, None]
    return y.reshape(B, C, 16, 16, 16)


def run(x, w_qkv, w_out, b_out, trace=False):
    from concourse.bass_utils import run_bass_kernel_spmd
    nc = _get_nc()
    in_maps = _make_in_maps(np.asarray(x), np.asarray(w_qkv), np.asarray(w_out))
    res = run_bass_kernel_spmd(nc, in_maps, core_ids=list(range(8)),
                               trace=trace)
    return _gather(res.results, np.asarray(b_out)), res


def kernel(x, w_qkv, w_out, b_out):
    y, _ = run(x, w_qkv, w_out, b_out)
    return y
